# revision 1
# baseline (speedup 1.0000x reference)
"""Multi-head attention with RoPE (LLaMA-style) on 8 Trainium2 NeuronCores.

Head-parallel tensor parallelism: each core computes 2 of 16 heads
(projections + flash-style attention), then an AllToAll reshards from
head-parallel to sequence-parallel so each core applies the full output
projection to its own 512 rows.  The host concatenates the 8 row-slices.

Self-contained: hardcodes B=1, S=4096, D=1024, H=16, HD=64, 8 cores.
"""

import sys
import types

import ml_dtypes
import numpy as np

B, S, D, H, HD = 1, 4096, 1024, 16, 64
HALF = HD // 2
NC = 8                    # cores
HPC = H // NC             # heads per core (2)
CPC = HPC * HD            # qkv dims per core (128)
SPC = S // NC             # output rows per core (512)
QCH = 512                 # query chunk (free dim of scores matmuls)
KCH = 128                 # key chunk (partition dim of scores matmuls)
NQC = S // QCH            # 8 query chunks
NKC = S // KCH            # 32 key chunks
P = 128
KC = D // P               # 8 contraction chunks for projections


def _install_ntff_shim():
    """antenv.axon_hooks isn't injected in this image; recreate it so
    run_bass_kernel_spmd(trace=True) can capture NTFF profiles."""
    if "antenv.axon_hooks" in sys.modules:
        return
    try:
        from trn_agent_boot.trn_boot import _ntff_profile_via_ctypes

        hook = _ntff_profile_via_ctypes("/opt/axon/libaxon_pjrt.so")
    except Exception:
        hook = None
    mod = types.ModuleType("antenv.axon_hooks")
    mod.get_axon_ntff_profile_hook = lambda: hook
    sys.modules["antenv.axon_hooks"] = mod


_install_ntff_shim()

import concourse.bacc as bacc  # noqa: E402
import concourse.mybir as mybir  # noqa: E402
import concourse.tile as tile  # noqa: E402
from concourse.bass_utils import run_bass_kernel_spmd  # noqa: E402

F32 = mybir.dt.float32
BF16 = mybir.dt.bfloat16
AX = mybir.AluOpType

_BUILD_CACHE: dict = {}


def _build(mask_mode: str):
    """Build the per-core Bass program.  mask_mode: causal | none | general."""
    if mask_mode in _BUILD_CACHE:
        return _BUILD_CACHE[mask_mode]

    nc = bacc.Bacc("TRN2", target_bir_lowering=False, debug=False, num_devices=NC)

    xbf = nc.dram_tensor("xbf", [S, D], BF16, kind="ExternalInput")
    wqkvT = nc.dram_tensor("wqkvT", [D, 3 * CPC], BF16, kind="ExternalInput")
    # per-core slice of wo_w.T (rows = this core's head dims)
    woT = nc.dram_tensor("woT", [CPC, D], BF16, kind="ExternalInput")
    # trig_a rows: [cos(32) sin(32) cos(32) sin(32)], trig_b: [sin cos sin cos]
    trig_a = nc.dram_tensor("trig_a", [P, S], F32, kind="ExternalInput")
    trig_b = nc.dram_tensor("trig_b", [P, S], F32, kind="ExternalInput")
    qkb = nc.dram_tensor("qkb", [P, 2], F32, kind="ExternalInput")
    ident = nc.dram_tensor("ident", [P, P], BF16, kind="ExternalInput")
    vbb = nc.dram_tensor("vbb", [P, CPC], F32, kind="ExternalInput")
    tri = None
    maskT = None
    if mask_mode == "causal":
        tri = nc.dram_tensor("tri", [KCH, KCH], F32, kind="ExternalInput")
    elif mask_mode == "general":
        maskT = nc.dram_tensor("maskT", [S, S], F32, kind="ExternalInput")
    # partial output (full rows; host sums the 8 per-core partials)
    y_out = nc.dram_tensor("y", [S, D], F32, kind="ExternalOutput")

    causal = mask_mode == "causal"

    with tile.TileContext(nc) as tc:
        with tc.tile_pool(name="consts", bufs=1) as cpool:
            # persistent activations + attention/wo constants
            qT_sb = cpool.tile([P, S], BF16)  # [2 heads x 64 dims, s] (pair-split)
            kT_sb = cpool.tile([P, S], BF16)
            v_sb = cpool.tile([P, NKC, 2 * (HD + 1)], BF16)  # [s%128, s//128, hd|1]
            attnT_sb = cpool.tile([P, S], BF16)
            woT_sb = cpool.tile([CPC, D], BF16)
            nc.sync.dma_start(out=woT_sb[:], in_=woT.ap())
            if causal:
                tri_sb = cpool.tile([KCH, KCH], F32)
                nc.sync.dma_start(out=tri_sb[:], in_=tri.ap())
            for h in range(HPC):
                col = h * (HD + 1) + HD
                nc.gpsimd.memset(v_sb[:, :, col : col + 1], 1.0)

            # ---------------- projections + RoPE ----------------
            with (
                tc.tile_pool(name="phA", bufs=1) as apool,
                tc.tile_pool(name="pq", bufs=1, space="PSUM") as pq_pool,
                tc.tile_pool(name="pk", bufs=1, space="PSUM") as pk_pool,
                tc.tile_pool(name="pv", bufs=2, space="PSUM") as pv_pool,
                tc.tile_pool(name="pr", bufs=2, space="PSUM") as pr_pool,
                tc.tile_pool(name="pt", bufs=2, space="PSUM") as pt_pool,
                tc.tile_pool(name="rtmp", bufs=4) as rtmp_pool,
                tc.tile_pool(name="xr", bufs=3) as xr_pool,
            ):
                trig_a_sb = apool.tile([P, S], F32)
                nc.sync.dma_start(out=trig_a_sb[:], in_=trig_a.ap())
                trig_b_sb = apool.tile([P, S], F32)
                nc.sync.dma_start(out=trig_b_sb[:], in_=trig_b.ap())
                qkb_sb = apool.tile([P, 2], F32)
                nc.sync.dma_start(out=qkb_sb[:], in_=qkb.ap())
                vbb_sb = apool.tile([P, CPC], F32)
                nc.sync.dma_start(out=vbb_sb[:], in_=vbb.ap())
                w_sb = apool.tile([P, KC, 3 * CPC], BF16)
                nc.sync.dma_start(
                    out=w_sb[:], in_=wqkvT.ap().rearrange("(a p) c -> p a c", p=P)
                )
                ident_sb = apool.tile([P, P], BF16)
                nc.sync.dma_start(out=ident_sb[:], in_=ident.ap())
                # x transposed via PE: [128 (dim within chunk), kc, s]
                # (dma_start_transpose hangs on HW under this runtime path)
                xT_sb = apool.tile([P, KC, S], BF16)
                for rc in range(S // P):
                    xr = xr_pool.tile([P, D], BF16)
                    nc.sync.dma_start(
                        out=xr[:], in_=xbf.ap()[rc * P : (rc + 1) * P, :]
                    )
                    ptt = pt_pool.tile([P, KC, P], BF16)
                    for kc in range(KC):
                        nc.tensor.transpose(
                            ptt[:, kc, :], xr[:, kc * P : (kc + 1) * P], ident_sb[:]
                        )
                    nc.scalar.copy(
                        xT_sb[:, :, rc * P : (rc + 1) * P], ptt[:]
                    )

                for sc in range(NQC):
                    ssl = slice(sc * QCH, (sc + 1) * QCH)
                    psq = pq_pool.tile([P, QCH], F32)
                    psk = pk_pool.tile([P, QCH], F32)
                    for kc in range(KC):
                        nc.tensor.matmul(
                            psq[:],
                            lhsT=w_sb[:, kc, 0:CPC],
                            rhs=xT_sb[:, kc, ssl],
                            start=(kc == 0),
                            stop=(kc == KC - 1),
                        )
                    for kc in range(KC):
                        nc.tensor.matmul(
                            psk[:],
                            lhsT=w_sb[:, kc, CPC : 2 * CPC],
                            rhs=xT_sb[:, kc, ssl],
                            start=(kc == 0),
                            stop=(kc == KC - 1),
                        )
                    # RoPE + bias, PSUM -> bf16 SBUF
                    # SB+SB operand pairs must share a base partition, so the
                    # cross-half operand of each combine goes through PSUM.
                    for dst, ps, bcol in ((qT_sb, psq, 0), (kT_sb, psk, 1)):
                        for h in range(HPC):
                            r0 = slice(64 * h, 64 * h + 32)       # x0 rows
                            r1 = slice(64 * h + 32, 64 * h + 64)  # x1 rows
                            t = rtmp_pool.tile([P, QCH], F32)
                            u = pr_pool.tile([P, QCH], F32)
                            # t[r0] = (x0+b)*cos ; t[r1] = (x1+b)*cos
                            nc.vector.scalar_tensor_tensor(
                                t[r0, :], ps[r0, :], qkb_sb[r0, bcol : bcol + 1],
                                trig_a_sb[r0, ssl], op0=AX.add, op1=AX.mult,
                            )
                            nc.vector.scalar_tensor_tensor(
                                t[r1, :], ps[r1, :], qkb_sb[r1, bcol : bcol + 1],
                                trig_b_sb[r1, ssl], op0=AX.add, op1=AX.mult,
                            )
                            # u[r1] = (x1+b)*sin ; u[r0] = (x0+b)*sin   (PSUM)
                            nc.vector.scalar_tensor_tensor(
                                u[r1, :], ps[r1, :], qkb_sb[r1, bcol : bcol + 1],
                                trig_a_sb[r1, ssl], op0=AX.add, op1=AX.mult,
                            )
                            nc.vector.scalar_tensor_tensor(
                                u[r0, :], ps[r0, :], qkb_sb[r0, bcol : bcol + 1],
                                trig_b_sb[r0, ssl], op0=AX.add, op1=AX.mult,
                            )
                            # o0 = t[r0] - u[r1] ; o1 = t[r1] + u[r0]
                            nc.vector.tensor_sub(dst[r0, ssl], t[r0, :], u[r1, :])
                            nc.vector.tensor_add(dst[r1, ssl], t[r1, :], u[r0, :])
                    # v projection: [s rows, c dims]
                    for j4 in range(4):
                        sb = sc * 4 + j4
                        psv = pv_pool.tile([P, CPC], F32)
                        for kc in range(KC):
                            nc.tensor.matmul(
                                psv[:],
                                lhsT=xT_sb[:, kc, sb * P : (sb + 1) * P],
                                rhs=w_sb[:, kc, 2 * CPC : 3 * CPC],
                                start=(kc == 0),
                                stop=(kc == KC - 1),
                            )
                        for h in range(HPC):
                            nc.vector.tensor_add(
                                v_sb[:, sb, h * (HD + 1) : h * (HD + 1) + HD],
                                psv[:, h * HD : (h + 1) * HD],
                                vbb_sb[:, h * HD : (h + 1) * HD],
                            )

            # ---------------- attention ----------------
            # q-chunks processed in pairs so the stationary operands (kT
            # chunk for QK, v chunk for PV) are shared back-to-back on PE.
            def n_kc_of(qc):
                return 4 * (qc + 1) if causal else NKC

            with (
                tc.tile_pool(name="ps", bufs=4, space="PSUM") as ps_pool,
                tc.tile_pool(name="ppv", bufs=4, space="PSUM") as ppv_pool,
                tc.tile_pool(name="probs", bufs=8) as probs_pool,
                tc.tile_pool(name="norm", bufs=4) as norm_pool,
                tc.tile_pool(name="mload", bufs=4) as mload_pool,
            ):
                for qp in range(0, NQC, 2):
                    qcs = tuple(qc for qc in (qp, qp + 1) if qc < NQC)
                    for h in range(HPC):
                        hr = slice(64 * h, 64 * h + 64)
                        ppvs = {
                            qc: ppv_pool.tile(
                                [HD + 1, QCH], F32, name="ppv", tag="ppv"
                            )
                            for qc in qcs
                        }
                        for j in range(n_kc_of(qcs[-1])):
                            for qc in qcs:
                                if j >= n_kc_of(qc):
                                    continue
                                ppv = ppvs[qc]
                                lo = max(0, KCH * j - QCH * qc) if causal else 0
                                ps = ps_pool.tile([P, QCH], F32)
                                nc.tensor.matmul(
                                    ps[:, lo:QCH],
                                    lhsT=kT_sb[hr, j * KCH : (j + 1) * KCH],
                                    rhs=qT_sb[hr, qc * QCH + lo : (qc + 1) * QCH],
                                    start=True,
                                    stop=True,
                                )
                                if mask_mode == "general":
                                    mt = mload_pool.tile([KCH, QCH], F32)
                                    nc.sync.dma_start(
                                        out=mt[:],
                                        in_=maskT.ap()[
                                            j * KCH : (j + 1) * KCH,
                                            qc * QCH : (qc + 1) * QCH,
                                        ],
                                    )
                                    nc.vector.tensor_add(ps[:], ps[:], mt[:])
                                pt = probs_pool.tile([P, QCH], BF16)
                                if causal and KCH * j >= QCH * qc:
                                    # diagonal 128x128 sub-tile needs masking
                                    nc.vector.tensor_add(
                                        ps[:, lo : lo + KCH],
                                        ps[:, lo : lo + KCH],
                                        tri_sb[:],
                                    )
                                    if lo > 0:
                                        nc.gpsimd.memset(pt[:, 0:lo], 0.0)
                                nc.scalar.activation(
                                    pt[:, lo:QCH],
                                    ps[:, lo:QCH],
                                    mybir.ActivationFunctionType.Exp,
                                    scale=0.125,
                                )
                                nc.tensor.matmul(
                                    ppv[:, lo:QCH],
                                    lhsT=v_sb[
                                        :, j, h * (HD + 1) : (h + 1) * (HD + 1)
                                    ],
                                    rhs=pt[:, lo:QCH],
                                    start=(j == 0),
                                    stop=(j == n_kc_of(qc) - 1),
                                    skip_group_check=True,
                                )
                        for qc in qcs:
                            ppv = ppvs[qc]
                            rec = norm_pool.tile([1, QCH], F32)
                            nc.vector.reciprocal(rec[:], ppv[HD : HD + 1, :])
                            bc = norm_pool.tile([P, QCH], F32)
                            nc.gpsimd.partition_broadcast(bc[:], rec[:])
                            nc.vector.tensor_mul(
                                attnT_sb[hr, qc * QCH : (qc + 1) * QCH],
                                ppv[0:HD, :],
                                bc[hr, :],
                            )

            # ---------------- output projection (partial; host sums) -------
            # wo_b is added on the host during the partial sum.
            with (
                tc.tile_pool(name="py", bufs=4, space="PSUM") as py_pool,
                tc.tile_pool(name="ysb", bufs=4) as y_pool,
            ):
                for sb in range(S // P):
                    for nn in range(D // 512):
                        nsl = slice(nn * 512, (nn + 1) * 512)
                        psy = py_pool.tile([P, 512], F32)
                        nc.tensor.matmul(
                            psy[:],
                            lhsT=attnT_sb[:, sb * P : (sb + 1) * P],
                            rhs=woT_sb[:, nsl],
                            start=True,
                            stop=True,
                        )
                        ysb = y_pool.tile([P, 512], F32)
                        if (sb * 2 + nn) % 2 == 0:
                            nc.vector.tensor_copy(ysb[:], psy[:])
                        else:
                            nc.scalar.copy(ysb[:], psy[:])
                        nc.sync.dma_start(
                            out=y_out.ap()[sb * P : (sb + 1) * P, nsl], in_=ysb[:]
                        )

    nc.compile()
    _BUILD_CACHE[mask_mode] = nc
    return nc


def _detect_mask_mode(mask: np.ndarray):
    m = np.asarray(mask, np.float32).reshape(S, S)
    if not m.any():
        return "none", 0.0, m
    mval = float(m[0, 1])
    if mval < -1e8 and np.array_equal(
        m, np.triu(np.full((S, S), mval, np.float32), 1)
    ):
        return "causal", mval, m
    return "general", 0.0, m


def kernel(
    x, start_pos, freqs_cos, freqs_sin, mask,
    wq_w, wq_b, wk_w, wk_b, wv_w, wv_b, wo_w, wo_b,
):
    x = np.asarray(x, np.float32).reshape(S, D)
    freqs_cos = np.asarray(freqs_cos, np.float32)
    freqs_sin = np.asarray(freqs_sin, np.float32)
    mask_mode, mval, m2d = _detect_mask_mode(np.asarray(mask))

    # pair-split permutation within each head: [0,2,..,62, 1,3,..,63]
    perm1 = np.concatenate([np.arange(0, HD, 2), np.arange(1, HD, 2)])
    perm = np.concatenate([perm1 + h * HD for h in range(HPC)])

    xbf = x.astype(ml_dtypes.bfloat16)

    # trig tiles: rows [0:32,32:64,64:96,96:128] =
    #   trig_a: cosT sinT cosT sinT ; trig_b: sinT cosT sinT cosT
    cosT = np.ascontiguousarray(freqs_cos.T)  # [32, S]
    sinT = np.ascontiguousarray(freqs_sin.T)
    trig_a = np.concatenate([cosT, sinT, cosT, sinT], axis=0).astype(np.float32)
    trig_b = np.concatenate([sinT, cosT, sinT, cosT], axis=0).astype(np.float32)

    woT_full = np.ascontiguousarray(np.asarray(wo_w, np.float32).T)

    tri_np = None
    if mask_mode == "causal":
        kk = np.arange(KCH)[:, None]
        qq = np.arange(KCH)[None, :]
        tri_np = np.where(kk > qq, np.float32(8.0 * mval), np.float32(0.0)).astype(
            np.float32
        )
    maskT_np = None
    if mask_mode == "general":
        maskT_np = np.ascontiguousarray((8.0 * m2d).T.astype(np.float32))

    in_maps = []
    for c in range(NC):
        rows = slice(c * CPC, (c + 1) * CPC)
        wq_s = np.asarray(wq_w, np.float32)[rows, :][perm, :]
        wk_s = np.asarray(wk_w, np.float32)[rows, :][perm, :]
        wv_s = np.asarray(wv_w, np.float32)[rows, :]
        wqkvT = np.concatenate([wq_s.T, wk_s.T, wv_s.T], axis=1).astype(
            ml_dtypes.bfloat16
        )
        qb = np.asarray(wq_b, np.float32)[rows][perm]
        kb = np.asarray(wk_b, np.float32)[rows][perm]
        vb = np.asarray(wv_b, np.float32)[rows]
        im = {
            "xbf": xbf,
            "wqkvT": np.ascontiguousarray(wqkvT),
            "woT": np.ascontiguousarray(woT_full[rows, :]).astype(ml_dtypes.bfloat16),
            "trig_a": trig_a,
            "trig_b": trig_b,
            "qkb": np.stack([qb, kb], axis=1).astype(np.float32),
            "ident": np.eye(P, dtype=ml_dtypes.bfloat16),
            "vbb": np.broadcast_to(vb, (P, CPC)).copy(),
        }
        if mask_mode == "causal":
            im["tri"] = tri_np
        elif mask_mode == "general":
            im["maskT"] = maskT_np
        in_maps.append(im)

    nc = _build(mask_mode)
    res = run_bass_kernel_spmd(nc, in_maps, list(range(NC)))
    y = np.zeros((S, D), np.float64)
    for c in range(NC):
        y += res.results[c]["y"].astype(np.float64)
    y += np.asarray(wo_b, np.float64)
    return y.reshape(B, S, D).astype(np.float32)



# revision 22
# speedup vs baseline: 1.6829x; 1.6829x over previous
"""Multi-head attention with RoPE (LLaMA-style) on 8 Trainium2 NeuronCores.

Head-parallel tensor parallelism: each core computes 2 of 16 heads
(projections + flash-style attention) and a partial output projection;
the host sums the 8 per-core partials.

Fused single-pass structure per core: for each 512-row chunk sc we
stream x^T (pre-transposed on host), project q/k/v, apply RoPE with
full-tile vector ops, then run attention for the *previous* chunk so
projection matmuls fill the PE while the scalar engine drains exp's.
The two heads' score matmuls use disjoint PE row groups (contraction
64 at base partitions 0/64) so they run concurrently, and each j-chunk's
scores for both heads land in one [128, 2, 512] PSUM group consumed by
a single batched exp ACTIVATE.

Self-contained: hardcodes B=1, S=4096, D=1024, H=16, HD=64, 8 cores.
"""

import sys
import types

import ml_dtypes
import numpy as np

B, S, D, H, HD = 1, 4096, 1024, 16, 64
HALF = HD // 2
NC = 8                    # cores
HPC = H // NC             # heads per core (2)
CPC = HPC * HD            # qkv dims per core (128)
QCH = 512                 # query chunk (free dim of scores matmuls)
KCH = 128                 # key chunk (partition dim of scores matmuls)
NQC = S // QCH            # 8 query chunks
NKC = S // KCH            # 32 key chunks
P = 128
KC = D // P               # 8 contraction chunks for projections
VW = HD + 1               # v columns per head (64 dims + ones row)


def _install_ntff_shim():
    """antenv.axon_hooks isn't injected in this image; recreate it so
    run_bass_kernel_spmd(trace=True) can capture NTFF profiles."""
    if "antenv.axon_hooks" in sys.modules:
        return
    try:
        from trn_agent_boot.trn_boot import _ntff_profile_via_ctypes

        hook = _ntff_profile_via_ctypes("/opt/axon/libaxon_pjrt.so")
    except Exception:
        hook = None
    mod = types.ModuleType("antenv.axon_hooks")
    mod.get_axon_ntff_profile_hook = lambda: hook
    sys.modules["antenv.axon_hooks"] = mod


_install_ntff_shim()

import concourse.bacc as bacc  # noqa: E402
import concourse.mybir as mybir  # noqa: E402
import concourse.tile as tile  # noqa: E402
from concourse.bass_utils import run_bass_kernel_spmd  # noqa: E402

F32 = mybir.dt.float32
BF16 = mybir.dt.bfloat16
AX = mybir.AluOpType

_BUILD_CACHE: dict = {}


def _build(mask_mode: str, debug: bool = False):
    """Build the per-core Bass program.  mask_mode: causal | none | general."""
    key = (mask_mode, debug)
    if key in _BUILD_CACHE:
        return _BUILD_CACHE[key]

    nc = bacc.Bacc("TRN2", target_bir_lowering=False, debug=False, num_devices=NC)

    # x^T pre-chunked on host: [p, sc, kc, t] = x[sc*512+t, kc*128+p]
    xtr = nc.dram_tensor("xtr", [P, NQC, KC, QCH], BF16, kind="ExternalInput")
    wqkvT = nc.dram_tensor("wqkvT", [D, 3 * CPC], BF16, kind="ExternalInput")
    # per-core slice of wo_w.T (rows = this core's head dims)
    woT = nc.dram_tensor("woT", [CPC, D], BF16, kind="ExternalInput")
    # trig rows replicated per 32-row group: trigC = [cosT]*4,
    # trigSN = [-sinT, sinT, -sinT, sinT] (sign folded for the rope combine)
    trigC = nc.dram_tensor("trigC", [P, S], F32, kind="ExternalInput")
    trigSN = nc.dram_tensor("trigSN", [P, S], F32, kind="ExternalInput")
    qkb = nc.dram_tensor("qkb", [P, 2], F32, kind="ExternalInput")
    # qkb with 32-row halves swapped inside each 64-row head block
    qkbs = nc.dram_tensor("qkbs", [P, 2], F32, kind="ExternalInput")
    vbb = nc.dram_tensor("vbb", [P, CPC], F32, kind="ExternalInput")
    tri2 = None
    maskT = None
    if mask_mode == "causal":
        # [128, 2, 128]: the same 128x128 additive causal block for each head
        tri2 = nc.dram_tensor("tri2", [KCH, 2, KCH], F32, kind="ExternalInput")
    elif mask_mode == "general":
        maskT = nc.dram_tensor("maskT", [S, S], F32, kind="ExternalInput")
    # partial output (full rows; host sums the 8 per-core partials)
    y_out = nc.dram_tensor("y", [S, D], F32, kind="ExternalOutput")
    dbg = {}
    if debug:
        dbg["qt"] = nc.dram_tensor("dbg_qt", [P, QCH], BF16, kind="ExternalOutput")
        dbg["kt"] = nc.dram_tensor("dbg_kt", [P, QCH], BF16, kind="ExternalOutput")
        dbg["v"] = nc.dram_tensor("dbg_v", [P, HPC * VW], BF16, kind="ExternalOutput")
        dbg["pt"] = nc.dram_tensor(
            "dbg_pt", [P, HPC, QCH], BF16, kind="ExternalOutput"
        )
        dbg["ppv"] = nc.dram_tensor("dbg_ppv", [P, QCH], F32, kind="ExternalOutput")
        dbg["rec"] = nc.dram_tensor("dbg_rec", [1, QCH], F32, kind="ExternalOutput")
        dbg["at"] = nc.dram_tensor("dbg_at", [P, QCH], BF16, kind="ExternalOutput")

    causal = mask_mode == "causal"

    def n_j_of(qc):
        return 4 * (qc + 1) if causal else NKC

    with tile.TileContext(nc) as tc:
        with tc.tile_pool(name="consts", bufs=1) as cpool:
            kT_sb = cpool.tile([P, S], BF16)           # keys^T, rope'd
            v_sb = cpool.tile([P, NKC, HPC * VW], BF16)  # [s%128, s//128, h*(hd|1)]
            w_sb = cpool.tile([P, KC, 3 * CPC], BF16)
            woT_sb = cpool.tile([CPC, D], BF16)
            qkb_sb = cpool.tile([P, 2], F32)
            qkbs_sb = cpool.tile([P, 2], F32)
            vbb_sb = cpool.tile([P, CPC], F32)
            nc.sync.dma_start(
                out=w_sb[:], in_=wqkvT.ap().rearrange("(a p) c -> p a c", p=P)
            )
            nc.sync.dma_start(out=woT_sb[:], in_=woT.ap())
            nc.sync.dma_start(out=qkb_sb[:], in_=qkb.ap())
            nc.sync.dma_start(out=qkbs_sb[:], in_=qkbs.ap())
            nc.sync.dma_start(out=vbb_sb[:], in_=vbb.ap())
            tri_sb = None
            if causal:
                tri_sb = cpool.tile([KCH, 2, KCH], F32)
                nc.sync.dma_start(out=tri_sb[:], in_=tri2.ap())
            for h in range(HPC):
                col = h * VW + HD
                nc.gpsimd.memset(v_sb[:, :, col : col + 1], 1.0)

            with (
                tc.tile_pool(name="xr", bufs=3) as xr_pool,
                tc.tile_pool(name="tc_", bufs=2) as tc_pool,
                tc.tile_pool(name="ts_", bufs=2) as ts_pool,
                tc.tile_pool(name="pps", bufs=2, space="PSUM") as pps_pool,
                tc.tile_pool(name="scp", bufs=2, space="PSUM") as sc_pool,
                tc.tile_pool(name="ppv", bufs=2, space="PSUM") as ppv_pool,
                tc.tile_pool(name="tt", bufs=2) as t_pool,
                tc.tile_pool(name="uu", bufs=2) as u_pool,
                tc.tile_pool(name="qT", bufs=2) as qT_pool,
                tc.tile_pool(name="pt", bufs=4) as pt_pool,
                tc.tile_pool(name="rec", bufs=2) as rec_pool,
                tc.tile_pool(name="lnz", bufs=2) as lnz_pool,
                tc.tile_pool(name="rec2", bufs=2) as rec2_pool,
                tc.tile_pool(name="bc", bufs=2) as bc_pool,
                tc.tile_pool(name="at", bufs=2) as at_pool,
                tc.tile_pool(name="ysb", bufs=4) as ysb_pool,
                tc.tile_pool(name="mload", bufs=4) as mload_pool,
            ):
                qTs = {}

                def proj(sc):
                    ssl = slice(sc * QCH, (sc + 1) * QCH)
                    xr = xr_pool.tile([P, KC, QCH], BF16, name="xr")
                    nc.sync.dma_start(out=xr[:], in_=xtr.ap()[:, sc, :, :])
                    tgc = tc_pool.tile([P, QCH], F32, name="tgc")
                    nc.sync.dma_start(out=tgc[:], in_=trigC.ap()[:, ssl])
                    tgsn = ts_pool.tile([P, QCH], F32, name="tgsn")
                    nc.sync.dma_start(out=tgsn[:], in_=trigSN.ap()[:, ssl])

                    # q and k projections + RoPE
                    qTc = qT_pool.tile([P, QCH], BF16, name="qTc")
                    qTs[sc] = qTc
                    for idx, dst in ((0, qTc), (1, kT_sb)):
                        ps = pps_pool.tile([P, QCH], F32, name="pps")
                        for kc in range(KC):
                            nc.tensor.matmul(
                                ps[:],
                                lhsT=w_sb[:, kc, idx * CPC : (idx + 1) * CPC],
                                rhs=xr[:, kc, :],
                                start=(kc == 0),
                                stop=(kc == KC - 1),
                            )
                        bcol = qkb_sb[:, idx : idx + 1]
                        bswc = qkbs_sb[:, idx : idx + 1]
                        # t = (ps+b)*cos (full tile).  usw = the cross-half
                        # sin product, written half-swapped so the final
                        # combine is one full-tile SB+SB add: the PSUM input
                        # reads the partner 32-row half (PSUM base may differ
                        # from the SBUF operands), the sign lives in tgsn.
                        t = t_pool.tile([P, QCH], BF16, name="t")
                        nc.vector.scalar_tensor_tensor(
                            t[:], ps[:], bcol, tgc[:], op0=AX.add, op1=AX.mult
                        )
                        usw = u_pool.tile([P, QCH], BF16, name="usw")
                        for h in range(HPC):
                            r0 = slice(64 * h, 64 * h + 32)
                            r1 = slice(64 * h + 32, 64 * h + 64)
                            nc.vector.scalar_tensor_tensor(
                                usw[r0, :], ps[r1, :], bswc[r0, :], tgsn[r0, :],
                                op0=AX.add, op1=AX.mult,
                            )
                            nc.vector.scalar_tensor_tensor(
                                usw[r1, :], ps[r0, :], bswc[r1, :], tgsn[r1, :],
                                op0=AX.add, op1=AX.mult,
                            )
                        osl = ssl if dst is kT_sb else slice(0, QCH)
                        nc.vector.tensor_add(dst[:, osl], t[:], usw[:])

                    if debug and sc == 0:
                        nc.sync.dma_start(out=dbg["qt"].ap(), in_=qTc[:])
                        nc.sync.dma_start(out=dbg["kt"].ap(), in_=kT_sb[:, 0:QCH])

                    # v projection: [s rows, c dims] per 128-row sub-block
                    for sb in range(QCH // P):
                        jb = sc * 4 + sb
                        psv = pps_pool.tile([P, CPC], F32, name="pps")
                        for kc in range(KC):
                            nc.tensor.matmul(
                                psv[:],
                                lhsT=xr[:, kc, sb * P : (sb + 1) * P],
                                rhs=w_sb[:, kc, 2 * CPC : 3 * CPC],
                                start=(kc == 0),
                                stop=(kc == KC - 1),
                            )
                        nc.vector.tensor_add(
                            v_sb[:, jb, :]
                            .rearrange("p (h c) -> p h c", h=HPC)[:, :, 0:HD],
                            psv.rearrange("p (h c) -> p h c", h=HPC),
                            vbb_sb.rearrange("p (h c) -> p h c", h=HPC),
                        )
                    if debug and sc == 0:
                        nc.sync.dma_start(out=dbg["v"].ap(), in_=v_sb[:, 0, :])

                def attn(qc):
                    qsl = slice(qc * QCH, (qc + 1) * QCH)
                    qTc = qTs.pop(qc)
                    n_j = n_j_of(qc)
                    ppvs = [
                        ppv_pool.tile([VW, QCH], F32, name="ppv", tag="ppv")
                        for _ in range(HPC)
                    ]
                    for j in range(n_j):
                        ps = sc_pool.tile([P, HPC, QCH], F32, name="ps")
                        for h in range(HPC):
                            hr = slice(64 * h, 64 * h + 64)
                            nc.tensor.matmul(
                                ps[:, h, :],
                                lhsT=kT_sb[hr, j * KCH : (j + 1) * KCH],
                                rhs=qTc[hr, :],
                                start=True,
                                stop=True,
                            )
                        lo = max(0, KCH * j - QCH * qc) if causal else 0
                        if causal and KCH * j >= QCH * qc:
                            nc.vector.tensor_add(
                                ps[:, :, lo : lo + KCH],
                                ps[:, :, lo : lo + KCH],
                                tri_sb[:],
                            )
                        if mask_mode == "general":
                            mt = mload_pool.tile([KCH, QCH], F32, name="mt")
                            nc.sync.dma_start(
                                out=mt[:],
                                in_=maskT.ap()[j * KCH : (j + 1) * KCH, qsl],
                            )
                            for h in range(HPC):
                                nc.vector.tensor_add(ps[:, h, :], ps[:, h, :], mt[:])
                        pt = pt_pool.tile([P, HPC, QCH], BF16, name="pt")
                        nc.scalar.activation(
                            pt[:], ps[:], mybir.ActivationFunctionType.Exp,
                            scale=0.125,
                        )
                        if debug and qc == 0 and j == 0:
                            nc.sync.dma_start(out=dbg["pt"].ap(), in_=pt[:])
                        for h in range(HPC):
                            nc.tensor.matmul(
                                ppvs[h][:, lo:QCH],
                                lhsT=v_sb[:, j, h * VW : (h + 1) * VW],
                                rhs=pt[:, h, lo:QCH],
                                start=(j == 0),
                                stop=(j == n_j - 1),
                                skip_group_check=True,
                            )
                    # normalize:  at[h] = ppv[h][0:HD] / ppv[h][HD]
                    # 1/Z via exp(-ln Z) on ScalarE: stock reciprocal is
                    # 8 cyc/elem on DVE and the custom approx op is broken
                    # on this runtime.  Both heads' denominators collect on
                    # partition 0 (clean -64 partition shift from PSUM).
                    den2 = rec_pool.tile([1, HPC, QCH], F32, name="den2")
                    for h in range(HPC):
                        nc.vector.tensor_copy(
                            den2[0:1, h, :], ppvs[h][HD : HD + 1, :]
                        )
                    lnz = lnz_pool.tile([1, HPC, QCH], F32, name="lnz")
                    nc.scalar.activation(
                        lnz[:], den2[:], mybir.ActivationFunctionType.Ln
                    )
                    rec2 = rec2_pool.tile([1, HPC, QCH], F32, name="rec2")
                    nc.scalar.activation(
                        rec2[:], lnz[:], mybir.ActivationFunctionType.Exp,
                        scale=-1.0,
                    )
                    at = at_pool.tile([P, QCH], BF16, name="at")
                    for h in range(HPC):
                        hr = slice(64 * h, 64 * h + 64)
                        bc = bc_pool.tile([P, QCH], F32, name="bc")
                        nc.gpsimd.partition_broadcast(bc[:], rec2[0:1, h, :])
                        nc.vector.tensor_mul(at[hr, :], ppvs[h][0:HD, :], bc[hr, :])
                        if debug and qc == 0 and h == 0:
                            ptmp = ysb_pool.tile([P, QCH], F32, name="ysb")
                            nc.vector.tensor_copy(ptmp[0:VW, :], ppvs[h][:])
                            nc.sync.dma_start(out=dbg["ppv"].ap(), in_=ptmp[:])
                            nc.sync.dma_start(out=dbg["rec"].ap(), in_=rec2[0:1, 0, :])
                    if debug and qc == 0:
                        nc.sync.dma_start(out=dbg["at"].ap(), in_=at[:])
                    # partial output projection for these 512 rows
                    for sb in range(QCH // P):
                        row0 = qc * QCH + sb * P
                        for nn in range(D // 512):
                            nsl = slice(nn * 512, (nn + 1) * 512)
                            psy = pps_pool.tile([P, 512], F32, name="pps")
                            nc.tensor.matmul(
                                psy[:],
                                lhsT=at[:, sb * P : (sb + 1) * P],
                                rhs=woT_sb[:, nsl],
                                start=True,
                                stop=True,
                            )
                            ysb = ysb_pool.tile([P, 512], F32, name="ysb")
                            nc.vector.tensor_copy(ysb[:], psy[:])
                            nc.sync.dma_start(
                                out=y_out.ap()[row0 : row0 + P, nsl], in_=ysb[:]
                            )

                for sc in range(NQC):
                    proj(sc)
                    if sc > 0:
                        attn(sc - 1)
                attn(NQC - 1)

    nc.compile()
    _BUILD_CACHE[key] = nc
    return nc


def _detect_mask_mode(mask: np.ndarray):
    m = np.asarray(mask, np.float32).reshape(S, S)
    if not m.any():
        return "none", 0.0, m
    mval = float(m[0, 1])
    if mval < -1e8 and np.array_equal(
        m, np.triu(np.full((S, S), mval, np.float32), 1)
    ):
        return "causal", mval, m
    return "general", 0.0, m


def kernel(
    x, start_pos, freqs_cos, freqs_sin, mask,
    wq_w, wq_b, wk_w, wk_b, wv_w, wv_b, wo_w, wo_b,
):
    x = np.asarray(x, np.float32).reshape(S, D)
    freqs_cos = np.asarray(freqs_cos, np.float32)
    freqs_sin = np.asarray(freqs_sin, np.float32)
    mask_mode, mval, m2d = _detect_mask_mode(np.asarray(mask))

    # pair-split permutation within each head: [0,2,..,62, 1,3,..,63]
    perm1 = np.concatenate([np.arange(0, HD, 2), np.arange(1, HD, 2)])
    perm = np.concatenate([perm1 + h * HD for h in range(HPC)])

    # x^T pre-chunked: [p, sc, kc, t] = x[sc*512+t, kc*128+p]
    xtr = np.ascontiguousarray(
        x.reshape(NQC, QCH, KC, P).transpose(3, 0, 2, 1)
    ).astype(ml_dtypes.bfloat16)

    # trig rows: each 32-row group is cos^T (trigC); trigSN carries the
    # combine sign: [-sin, +sin] per 64-row head block
    cosT = np.ascontiguousarray(freqs_cos.T)  # [32, S]
    sinT = np.ascontiguousarray(freqs_sin.T)
    trigC = np.concatenate([cosT] * 4, axis=0).astype(np.float32)
    trigSN = np.concatenate([-sinT, sinT] * 2, axis=0).astype(np.float32)

    woT_full = np.ascontiguousarray(np.asarray(wo_w, np.float32).T)

    tri2_np = None
    if mask_mode == "causal":
        kk = np.arange(KCH)[:, None]
        qq = np.arange(KCH)[None, :]
        tri1 = np.where(kk > qq, np.float32(8.0 * mval), np.float32(0.0)).astype(
            np.float32
        )
        tri2_np = np.ascontiguousarray(
            np.broadcast_to(tri1[:, None, :], (KCH, 2, KCH))
        )
    maskT_np = None
    if mask_mode == "general":
        maskT_np = np.ascontiguousarray((8.0 * m2d).T.astype(np.float32))

    in_maps = []
    for c in range(NC):
        rows = slice(c * CPC, (c + 1) * CPC)
        wq_s = np.asarray(wq_w, np.float32)[rows, :][perm, :]
        wk_s = np.asarray(wk_w, np.float32)[rows, :][perm, :]
        wv_s = np.asarray(wv_w, np.float32)[rows, :]
        wqkvT = np.concatenate([wq_s.T, wk_s.T, wv_s.T], axis=1).astype(
            ml_dtypes.bfloat16
        )
        qb = np.asarray(wq_b, np.float32)[rows][perm]
        kb = np.asarray(wk_b, np.float32)[rows][perm]
        vb = np.asarray(wv_b, np.float32)[rows]
        qkb_np = np.stack([qb, kb], axis=1).astype(np.float32)
        # swap the 32-row halves within each 64-row head block
        qkbs_np = np.ascontiguousarray(
            qkb_np.reshape(HPC, 2, 32, 2)[:, ::-1].reshape(P, 2)
        )
        im = {
            "xtr": xtr,
            "wqkvT": np.ascontiguousarray(wqkvT),
            "woT": np.ascontiguousarray(woT_full[rows, :]).astype(ml_dtypes.bfloat16),
            "trigC": trigC,
            "trigSN": trigSN,
            "qkb": qkb_np,
            "qkbs": qkbs_np,
            "vbb": np.broadcast_to(vb, (P, CPC)).copy(),
        }
        if mask_mode == "causal":
            im["tri2"] = tri2_np
        elif mask_mode == "general":
            im["maskT"] = maskT_np
        in_maps.append(im)

    nc = _build(mask_mode)
    res = run_bass_kernel_spmd(nc, in_maps, list(range(NC)))
    y = np.zeros((S, D), np.float64)
    for c in range(NC):
        y += res.results[c]["y"].astype(np.float64)
    y += np.asarray(wo_b, np.float64)
    return y.reshape(B, S, D).astype(np.float32)


# revision 25
# speedup vs baseline: 1.9782x; 1.1754x over previous
"""Multi-head attention with RoPE (LLaMA-style) on 8 Trainium2 NeuronCores.

Head-parallel tensor parallelism: each core computes 2 of 16 heads
(projections + flash-style attention) and a partial output projection;
the host sums the 8 per-core partials.

Fused single-pass structure per core: for each 512-row chunk sc we
stream x^T (pre-transposed on host), project q/k/v, apply RoPE with
full-tile vector ops, then run attention for the *previous* chunk so
projection matmuls fill the PE while the scalar engine drains exp's.
The two heads' score matmuls use disjoint PE row groups (contraction
64 at base partitions 0/64) so they run concurrently, and each j-chunk's
scores for both heads land in one [128, 2, 512] PSUM group consumed by
a single batched exp ACTIVATE.

Self-contained: hardcodes B=1, S=4096, D=1024, H=16, HD=64, 8 cores.
"""

import sys
import types

import ml_dtypes
import numpy as np

B, S, D, H, HD = 1, 4096, 1024, 16, 64
HALF = HD // 2
NC = 8                    # cores
HPC = H // NC             # heads per core (2)
CPC = HPC * HD            # qkv dims per core (128)
QCH = 512                 # query chunk (free dim of scores matmuls)
KCH = 128                 # key chunk (partition dim of scores matmuls)
NQC = S // QCH            # 8 query chunks
NKC = S // KCH            # 32 key chunks
P = 128
KC = D // P               # 8 contraction chunks for projections
VW = HD + 1               # v columns per head (64 dims + ones row)


def _install_ntff_shim():
    """antenv.axon_hooks isn't injected in this image; recreate it so
    run_bass_kernel_spmd(trace=True) can capture NTFF profiles."""
    if "antenv.axon_hooks" in sys.modules:
        return
    try:
        from trn_agent_boot.trn_boot import _ntff_profile_via_ctypes

        hook = _ntff_profile_via_ctypes("/opt/axon/libaxon_pjrt.so")
    except Exception:
        hook = None
    mod = types.ModuleType("antenv.axon_hooks")
    mod.get_axon_ntff_profile_hook = lambda: hook
    sys.modules["antenv.axon_hooks"] = mod


_install_ntff_shim()

import concourse.bacc as bacc  # noqa: E402
import concourse.mybir as mybir  # noqa: E402
import concourse.tile as tile  # noqa: E402
from concourse.bass_utils import run_bass_kernel_spmd  # noqa: E402


def _install_act_table_preference():
    """The act-table-load pass picks the first set containing each function,
    which alternates exp_and_others <-> natural_log and reloads tables every
    chunk.  Hiding Ln from the standalone natural_log set forces the picker
    onto natural_log_exp_and_others (contains BOTH Exp and Ln), so after one
    load every Exp/Ln activation hits the resident set.  Set ids still index
    the unmodified act_info.json list, so runtime behavior is unchanged."""
    if getattr(bacc, "_ant_act_tables_patched", False):
        return
    orig = bacc.get_activation_tables
    cache: dict = {}

    def patched(arch):
        if arch not in cache:
            t = dict(orig(arch))
            if "natural_log" in t and "natural_log_exp_and_others" in t:
                t["natural_log"] = t["natural_log"] - {
                    mybir.ActivationFunctionType.Ln
                }
            cache[arch] = t
        return cache[arch]

    bacc.get_activation_tables = patched
    bacc._ant_act_tables_patched = True


_install_act_table_preference()

F32 = mybir.dt.float32
BF16 = mybir.dt.bfloat16
AX = mybir.AluOpType

_BUILD_CACHE: dict = {}


def _build(mask_mode: str, debug: bool = False):
    """Build the per-core Bass program.  mask_mode: causal | none | general."""
    key = (mask_mode, debug)
    if key in _BUILD_CACHE:
        return _BUILD_CACHE[key]

    nc = bacc.Bacc("TRN2", target_bir_lowering=False, debug=False, num_devices=NC)

    # x^T pre-chunked on host: [p, sc, kc, t] = x[sc*512+t, kc*128+p]
    xtr = nc.dram_tensor("xtr", [P, NQC, KC, QCH], BF16, kind="ExternalInput")
    wqkvT = nc.dram_tensor("wqkvT", [D, 3 * CPC], BF16, kind="ExternalInput")
    # per-core slice of wo_w.T (rows = this core's head dims)
    woT = nc.dram_tensor("woT", [CPC, D], BF16, kind="ExternalInput")
    # trig rows replicated per 32-row group: trigC = [cosT]*4,
    # trigSN = [-sinT, sinT, -sinT, sinT] (sign folded for the rope combine)
    trigC = nc.dram_tensor("trigC", [P, S], F32, kind="ExternalInput")
    trigSN = nc.dram_tensor("trigSN", [P, S], F32, kind="ExternalInput")
    qkb = nc.dram_tensor("qkb", [P, 2], F32, kind="ExternalInput")
    # qkb with 32-row halves swapped inside each 64-row head block
    qkbs = nc.dram_tensor("qkbs", [P, 2], F32, kind="ExternalInput")
    vbb = nc.dram_tensor("vbb", [P, CPC], F32, kind="ExternalInput")
    tri2 = None
    maskT = None
    if mask_mode == "causal":
        # [128, 2, 128]: the same 128x128 additive causal block for each head
        tri2 = nc.dram_tensor("tri2", [KCH, 2, KCH], F32, kind="ExternalInput")
    elif mask_mode == "general":
        maskT = nc.dram_tensor("maskT", [S, S], F32, kind="ExternalInput")
    # partial output (full rows; host sums the 8 per-core partials)
    y_out = nc.dram_tensor("y", [S, D], F32, kind="ExternalOutput")
    dbg = {}
    if debug:
        dbg["qt"] = nc.dram_tensor("dbg_qt", [P, QCH], BF16, kind="ExternalOutput")
        dbg["kt"] = nc.dram_tensor("dbg_kt", [P, QCH], BF16, kind="ExternalOutput")
        dbg["v"] = nc.dram_tensor("dbg_v", [P, HPC * VW], BF16, kind="ExternalOutput")
        dbg["pt"] = nc.dram_tensor(
            "dbg_pt", [P, HPC, QCH], BF16, kind="ExternalOutput"
        )
        dbg["ppv"] = nc.dram_tensor("dbg_ppv", [P, QCH], F32, kind="ExternalOutput")
        dbg["rec"] = nc.dram_tensor("dbg_rec", [1, QCH], F32, kind="ExternalOutput")
        dbg["at"] = nc.dram_tensor("dbg_at", [P, QCH], BF16, kind="ExternalOutput")

    causal = mask_mode == "causal"

    def n_j_of(qc):
        return 4 * (qc + 1) if causal else NKC

    with tile.TileContext(nc) as tc:
        with tc.tile_pool(name="consts", bufs=1) as cpool:
            kT_sb = cpool.tile([P, S], BF16)           # keys^T, rope'd
            v_sb = cpool.tile([P, NKC, HPC * VW], BF16)  # [s%128, s//128, h*(hd|1)]
            w_sb = cpool.tile([P, KC, 3 * CPC], BF16)
            woT_sb = cpool.tile([CPC, D], BF16)
            qkb_sb = cpool.tile([P, 2], F32)
            qkbs_sb = cpool.tile([P, 2], F32)
            vbb_sb = cpool.tile([P, CPC], F32)
            nc.sync.dma_start(
                out=w_sb[:], in_=wqkvT.ap().rearrange("(a p) c -> p a c", p=P)
            )
            nc.sync.dma_start(out=woT_sb[:], in_=woT.ap())
            nc.sync.dma_start(out=qkb_sb[:], in_=qkb.ap())
            nc.sync.dma_start(out=qkbs_sb[:], in_=qkbs.ap())
            nc.sync.dma_start(out=vbb_sb[:], in_=vbb.ap())
            tri_sb = None
            if causal:
                tri_sb = cpool.tile([KCH, 2, KCH], F32)
                nc.sync.dma_start(out=tri_sb[:], in_=tri2.ap())
            for h in range(HPC):
                col = h * VW + HD
                nc.gpsimd.memset(v_sb[:, :, col : col + 1], 1.0)

            with (
                tc.tile_pool(name="xr", bufs=3) as xr_pool,
                tc.tile_pool(name="tc_", bufs=2) as tc_pool,
                tc.tile_pool(name="ts_", bufs=2) as ts_pool,
                tc.tile_pool(name="pps", bufs=2, space="PSUM") as pps_pool,
                tc.tile_pool(name="scp", bufs=2, space="PSUM") as sc_pool,
                tc.tile_pool(name="ppv", bufs=2, space="PSUM") as ppv_pool,
                tc.tile_pool(name="tt", bufs=2) as t_pool,
                tc.tile_pool(name="uu", bufs=2) as u_pool,
                tc.tile_pool(name="qT", bufs=2) as qT_pool,
                tc.tile_pool(name="pt", bufs=4) as pt_pool,
                tc.tile_pool(name="rec", bufs=2) as rec_pool,
                tc.tile_pool(name="lnz", bufs=2) as lnz_pool,
                tc.tile_pool(name="rec2", bufs=2) as rec2_pool,
                tc.tile_pool(name="bc", bufs=2) as bc_pool,
                tc.tile_pool(name="at", bufs=2) as at_pool,
                tc.tile_pool(name="ysb", bufs=4) as ysb_pool,
                tc.tile_pool(name="mload", bufs=4) as mload_pool,
            ):
                qTs = {}
                ats = {}

                def proj(sc):
                    ssl = slice(sc * QCH, (sc + 1) * QCH)
                    xr = xr_pool.tile([P, KC, QCH], BF16, name="xr")
                    nc.sync.dma_start(out=xr[:], in_=xtr.ap()[:, sc, :, :])
                    tgc = tc_pool.tile([P, QCH], F32, name="tgc")
                    nc.sync.dma_start(out=tgc[:], in_=trigC.ap()[:, ssl])
                    tgsn = ts_pool.tile([P, QCH], F32, name="tgsn")
                    nc.sync.dma_start(out=tgsn[:], in_=trigSN.ap()[:, ssl])

                    # q and k projections + RoPE
                    qTc = qT_pool.tile([P, QCH], BF16, name="qTc")
                    qTs[sc] = qTc
                    for idx, dst in ((0, qTc), (1, kT_sb)):
                        ps = pps_pool.tile([P, QCH], F32, name="pps")
                        for kc in range(KC):
                            nc.tensor.matmul(
                                ps[:],
                                lhsT=w_sb[:, kc, idx * CPC : (idx + 1) * CPC],
                                rhs=xr[:, kc, :],
                                start=(kc == 0),
                                stop=(kc == KC - 1),
                            )
                        bcol = qkb_sb[:, idx : idx + 1]
                        bswc = qkbs_sb[:, idx : idx + 1]
                        # t = (ps+b)*cos (full tile).  usw = the cross-half
                        # sin product, written half-swapped so the final
                        # combine is one full-tile SB+SB add: the PSUM input
                        # reads the partner 32-row half (PSUM base may differ
                        # from the SBUF operands), the sign lives in tgsn.
                        t = t_pool.tile([P, QCH], BF16, name="t")
                        nc.vector.scalar_tensor_tensor(
                            t[:], ps[:], bcol, tgc[:], op0=AX.add, op1=AX.mult
                        )
                        usw = u_pool.tile([P, QCH], BF16, name="usw")
                        for h in range(HPC):
                            r0 = slice(64 * h, 64 * h + 32)
                            r1 = slice(64 * h + 32, 64 * h + 64)
                            nc.vector.scalar_tensor_tensor(
                                usw[r0, :], ps[r1, :], bswc[r0, :], tgsn[r0, :],
                                op0=AX.add, op1=AX.mult,
                            )
                            nc.vector.scalar_tensor_tensor(
                                usw[r1, :], ps[r0, :], bswc[r1, :], tgsn[r1, :],
                                op0=AX.add, op1=AX.mult,
                            )
                        osl = ssl if dst is kT_sb else slice(0, QCH)
                        nc.vector.tensor_add(dst[:, osl], t[:], usw[:])

                    if debug and sc == 0:
                        nc.sync.dma_start(out=dbg["qt"].ap(), in_=qTc[:])
                        nc.sync.dma_start(out=dbg["kt"].ap(), in_=kT_sb[:, 0:QCH])

                    # v projection: [s rows, c dims] per 128-row sub-block
                    for sb in range(QCH // P):
                        jb = sc * 4 + sb
                        psv = pps_pool.tile([P, CPC], F32, name="pps")
                        for kc in range(KC):
                            nc.tensor.matmul(
                                psv[:],
                                lhsT=xr[:, kc, sb * P : (sb + 1) * P],
                                rhs=w_sb[:, kc, 2 * CPC : 3 * CPC],
                                start=(kc == 0),
                                stop=(kc == KC - 1),
                            )
                        nc.vector.tensor_add(
                            v_sb[:, jb, :]
                            .rearrange("p (h c) -> p h c", h=HPC)[:, :, 0:HD],
                            psv.rearrange("p (h c) -> p h c", h=HPC),
                            vbb_sb.rearrange("p (h c) -> p h c", h=HPC),
                        )
                    if debug and sc == 0:
                        nc.sync.dma_start(out=dbg["v"].ap(), in_=v_sb[:, 0, :])

                def attn(qc):
                    qsl = slice(qc * QCH, (qc + 1) * QCH)
                    qTc = qTs.pop(qc)
                    n_j = n_j_of(qc)
                    ppvs = [
                        ppv_pool.tile([VW, QCH], F32, name="ppv", tag="ppv")
                        for _ in range(HPC)
                    ]
                    for j in range(n_j):
                        ps = sc_pool.tile([P, HPC, QCH], F32, name="ps")
                        for h in range(HPC):
                            hr = slice(64 * h, 64 * h + 64)
                            nc.tensor.matmul(
                                ps[:, h, :],
                                lhsT=kT_sb[hr, j * KCH : (j + 1) * KCH],
                                rhs=qTc[hr, :],
                                start=True,
                                stop=True,
                            )
                        lo = max(0, KCH * j - QCH * qc) if causal else 0
                        if causal and KCH * j >= QCH * qc:
                            nc.vector.tensor_add(
                                ps[:, :, lo : lo + KCH],
                                ps[:, :, lo : lo + KCH],
                                tri_sb[:],
                            )
                        if mask_mode == "general":
                            mt = mload_pool.tile([KCH, QCH], F32, name="mt")
                            nc.sync.dma_start(
                                out=mt[:],
                                in_=maskT.ap()[j * KCH : (j + 1) * KCH, qsl],
                            )
                            for h in range(HPC):
                                nc.vector.tensor_add(ps[:, h, :], ps[:, h, :], mt[:])
                        pt = pt_pool.tile([P, HPC, QCH], BF16, name="pt")
                        nc.scalar.activation(
                            pt[:], ps[:], mybir.ActivationFunctionType.Exp,
                            scale=0.125,
                        )
                        if debug and qc == 0 and j == 0:
                            nc.sync.dma_start(out=dbg["pt"].ap(), in_=pt[:])
                        for h in range(HPC):
                            nc.tensor.matmul(
                                ppvs[h][:, lo:QCH],
                                lhsT=v_sb[:, j, h * VW : (h + 1) * VW],
                                rhs=pt[:, h, lo:QCH],
                                start=(j == 0),
                                stop=(j == n_j - 1),
                                skip_group_check=True,
                            )
                    # normalize:  at[h] = ppv[h][0:HD] / ppv[h][HD]
                    # 1/Z via exp(-ln Z) on ScalarE: stock reciprocal is
                    # 8 cyc/elem on DVE and the custom approx op is broken
                    # on this runtime.  Both heads' denominators collect on
                    # partition 0 (clean -64 partition shift from PSUM).
                    den2 = rec_pool.tile([1, HPC, QCH], F32, name="den2")
                    for h in range(HPC):
                        nc.vector.tensor_copy(
                            den2[0:1, h, :], ppvs[h][HD : HD + 1, :]
                        )
                    lnz = lnz_pool.tile([1, HPC, QCH], F32, name="lnz")
                    nc.scalar.activation(
                        lnz[:], den2[:], mybir.ActivationFunctionType.Ln
                    )
                    rec2 = rec2_pool.tile([1, HPC, QCH], F32, name="rec2")
                    nc.scalar.activation(
                        rec2[:], lnz[:], mybir.ActivationFunctionType.Exp,
                        scale=-1.0,
                    )
                    at = at_pool.tile([P, QCH], BF16, name="at")
                    for h in range(HPC):
                        hr = slice(64 * h, 64 * h + 64)
                        bc = bc_pool.tile([P, QCH], F32, name="bc")
                        nc.gpsimd.partition_broadcast(bc[:], rec2[0:1, h, :])
                        nc.vector.tensor_mul(at[hr, :], ppvs[h][0:HD, :], bc[hr, :])
                        if debug and qc == 0 and h == 0:
                            ptmp = ysb_pool.tile([P, QCH], F32, name="ysb")
                            nc.vector.tensor_copy(ptmp[0:VW, :], ppvs[h][:])
                            nc.sync.dma_start(out=dbg["ppv"].ap(), in_=ptmp[:])
                            nc.sync.dma_start(out=dbg["rec"].ap(), in_=rec2[0:1, 0, :])
                    if debug and qc == 0:
                        nc.sync.dma_start(out=dbg["at"].ap(), in_=at[:])
                    ats[qc] = at

                def wo(qc):
                    # partial output projection for these 512 rows; emitted a
                    # chunk late so the PE never stalls on the normalize chain
                    at = ats.pop(qc)
                    for sb in range(QCH // P):
                        row0 = qc * QCH + sb * P
                        for nn in range(D // 512):
                            nsl = slice(nn * 512, (nn + 1) * 512)
                            psy = pps_pool.tile([P, 512], F32, name="pps")
                            nc.tensor.matmul(
                                psy[:],
                                lhsT=at[:, sb * P : (sb + 1) * P],
                                rhs=woT_sb[:, nsl],
                                start=True,
                                stop=True,
                            )
                            ysb = ysb_pool.tile([P, 512], F32, name="ysb")
                            nc.vector.tensor_copy(ysb[:], psy[:])
                            nc.sync.dma_start(
                                out=y_out.ap()[row0 : row0 + P, nsl], in_=ysb[:]
                            )

                for sc in range(NQC):
                    proj(sc)
                    if sc > 0:
                        attn(sc - 1)
                    if sc > 1:
                        wo(sc - 2)
                attn(NQC - 1)
                wo(NQC - 2)
                wo(NQC - 1)

    nc.compile()
    _BUILD_CACHE[key] = nc
    return nc


def _detect_mask_mode(mask: np.ndarray):
    m = np.asarray(mask, np.float32).reshape(S, S)
    if not m.any():
        return "none", 0.0, m
    mval = float(m[0, 1])
    if mval < -1e8 and np.array_equal(
        m, np.triu(np.full((S, S), mval, np.float32), 1)
    ):
        return "causal", mval, m
    return "general", 0.0, m


def kernel(
    x, start_pos, freqs_cos, freqs_sin, mask,
    wq_w, wq_b, wk_w, wk_b, wv_w, wv_b, wo_w, wo_b,
):
    x = np.asarray(x, np.float32).reshape(S, D)
    freqs_cos = np.asarray(freqs_cos, np.float32)
    freqs_sin = np.asarray(freqs_sin, np.float32)
    mask_mode, mval, m2d = _detect_mask_mode(np.asarray(mask))

    # pair-split permutation within each head: [0,2,..,62, 1,3,..,63]
    perm1 = np.concatenate([np.arange(0, HD, 2), np.arange(1, HD, 2)])
    perm = np.concatenate([perm1 + h * HD for h in range(HPC)])

    # x^T pre-chunked: [p, sc, kc, t] = x[sc*512+t, kc*128+p]
    xtr = np.ascontiguousarray(
        x.reshape(NQC, QCH, KC, P).transpose(3, 0, 2, 1)
    ).astype(ml_dtypes.bfloat16)

    # trig rows: each 32-row group is cos^T (trigC); trigSN carries the
    # combine sign: [-sin, +sin] per 64-row head block
    cosT = np.ascontiguousarray(freqs_cos.T)  # [32, S]
    sinT = np.ascontiguousarray(freqs_sin.T)
    trigC = np.concatenate([cosT] * 4, axis=0).astype(np.float32)
    trigSN = np.concatenate([-sinT, sinT] * 2, axis=0).astype(np.float32)

    woT_full = np.ascontiguousarray(np.asarray(wo_w, np.float32).T)

    tri2_np = None
    if mask_mode == "causal":
        kk = np.arange(KCH)[:, None]
        qq = np.arange(KCH)[None, :]
        tri1 = np.where(kk > qq, np.float32(8.0 * mval), np.float32(0.0)).astype(
            np.float32
        )
        tri2_np = np.ascontiguousarray(
            np.broadcast_to(tri1[:, None, :], (KCH, 2, KCH))
        )
    maskT_np = None
    if mask_mode == "general":
        maskT_np = np.ascontiguousarray((8.0 * m2d).T.astype(np.float32))

    in_maps = []
    for c in range(NC):
        rows = slice(c * CPC, (c + 1) * CPC)
        wq_s = np.asarray(wq_w, np.float32)[rows, :][perm, :]
        wk_s = np.asarray(wk_w, np.float32)[rows, :][perm, :]
        wv_s = np.asarray(wv_w, np.float32)[rows, :]
        wqkvT = np.concatenate([wq_s.T, wk_s.T, wv_s.T], axis=1).astype(
            ml_dtypes.bfloat16
        )
        qb = np.asarray(wq_b, np.float32)[rows][perm]
        kb = np.asarray(wk_b, np.float32)[rows][perm]
        vb = np.asarray(wv_b, np.float32)[rows]
        qkb_np = np.stack([qb, kb], axis=1).astype(np.float32)
        # swap the 32-row halves within each 64-row head block
        qkbs_np = np.ascontiguousarray(
            qkb_np.reshape(HPC, 2, 32, 2)[:, ::-1].reshape(P, 2)
        )
        im = {
            "xtr": xtr,
            "wqkvT": np.ascontiguousarray(wqkvT),
            "woT": np.ascontiguousarray(woT_full[rows, :]).astype(ml_dtypes.bfloat16),
            "trigC": trigC,
            "trigSN": trigSN,
            "qkb": qkb_np,
            "qkbs": qkbs_np,
            "vbb": np.broadcast_to(vb, (P, CPC)).copy(),
        }
        if mask_mode == "causal":
            im["tri2"] = tri2_np
        elif mask_mode == "general":
            im["maskT"] = maskT_np
        in_maps.append(im)

    nc = _build(mask_mode)
    res = run_bass_kernel_spmd(nc, in_maps, list(range(NC)))
    y = np.zeros((S, D), np.float64)
    for c in range(NC):
        y += res.results[c]["y"].astype(np.float64)
    y += np.asarray(wo_b, np.float64)
    return y.reshape(B, S, D).astype(np.float32)
